# revision 1
# baseline (speedup 1.0000x reference)
"""Trainium2 Bass kernel for AttentionSTModule (dense transformer block).

Sharding: 8 cores = (batch b in {0,1}) x (query-quarter q in {0..3}).
Each core runs the full pre-attention pipeline (fusion MLP, LN1, K/V
projections) for its batch's 4096 tokens (4x replicated - cheap), but only
its own 1024 query tokens through attention + post-MLP.  No cross-core
communication: per-core inputs are token-rotated so "own" tokens are always
columns 0:1024 (SPMD program identical across cores).
"""

import functools
import numpy as np

B, C, T, H, W = 2, 128, 16, 16, 16
HW = H * W            # 256
N = HW * T            # 4096 tokens per batch
HEADS, DH = 8, 32
HID = HEADS * DH      # 256
MLP_H = 512
SCALE = DH ** -0.5
NCORES = 8
OWN = N // 4          # 1024 own query tokens per core
EPS = 1e-5


def _build(rep=1, variant="full"):
    import concourse.bass as bass
    import concourse.mybir as mybir
    import concourse.tile as tile
    from concourse import bacc
    from concourse.masks import make_identity
    from contextlib import ExitStack, nullcontext

    fp32 = mybir.dt.float32
    bf16 = mybir.dt.bfloat16
    AF = mybir.ActivationFunctionType
    ALU = mybir.AluOpType

    nc = bacc.Bacc("TRN2", target_bir_lowering=False, debug=False,
                   enable_asserts=False, num_devices=NCORES)

    # ---------------- DRAM I/O ----------------
    def din(name, shape):
        return nc.dram_tensor(name, shape, fp32, kind="ExternalInput")

    d_xfm = din("xfm", [C, N])          # feature-major x, token-rotated
    d_frow = din("frow", [1, N])        # frame-idx feature row
    d_w1a = din("w1a", [C, MLP_H])
    d_w1b = din("w1b", [1, MLP_H])
    d_b1t = din("b1t", [C, 4])          # fusion_b1 as [p, mh]
    d_w2 = din("w2", [C, 4, C])         # fusion_w2 k-tiled: [p, mh, c]
    d_b2 = din("b2", [1, C])
    d_ag = din("attn_g", [C, 1])
    d_ab = din("attn_b", [C, 1])
    d_wq = din("wq", [C, HID])
    d_wk = din("wk", [C, HID])
    d_wv = din("wv", [C, HID])
    d_wo = din("wo", [C, 2, C])         # wo k-tiled: [p, g, c]
    d_bo = din("bo", [C, 1])
    d_ng = din("norm_g", [C, 1])
    d_nb = din("norm_b", [C, 1])
    d_mw1 = din("mw1", [C, MLP_H])
    d_mw2 = din("mw2", [C, 4, C])       # mlp_w2 k-tiled
    d_mb1t = din("mb1t", [C, 4])
    d_mb2 = din("mb2", [1, C])
    d_ind = din("ind128", [C, C])       # [j, p] = (j == 32*(p//32))
    d_out = nc.dram_tensor("out", [OWN, C], fp32, kind="ExternalOutput")

    def bcast_ap(d, p=C):
        # broadcast a [1, F] DRAM row across p partitions
        a = d.ap()
        return bass.AP(tensor=a.tensor, offset=0, ap=[[0, p]] + a.ap[1:])

    with tile.TileContext(nc) as tc, ExitStack() as S:
        if rep > 1:
            S.enter_context(tc.For_i(0, rep, 1))
        sb = S.enter_context(tc.tile_pool(name="persist", bufs=1))
        scr = S.enter_context(tc.tile_pool(name="scratch", bufs=2))

        # ------------- load + cast weights -------------
        def load_cast(d, shape, name):
            t32 = scr.tile(shape, fp32, tag="ldtmp")
            nc.sync.dma_start(t32, d.ap())
            tb = sb.tile(shape, bf16, tag=name)
            nc.vector.tensor_copy(tb, t32)
            return tb

        w1a = load_cast(d_w1a, [C, MLP_H], "w1a")
        w1b = load_cast(d_w1b, [1, MLP_H], "w1b")
        w2 = load_cast(d_w2, [C, 4, C], "w2")
        wo = load_cast(d_wo, [C, 2, C], "wo")
        mw2 = load_cast(d_mw2, [C, 4, C], "mw2")
        ind = load_cast(d_ind, [C, C], "ind")

        # per-partition bias/gain tiles (fp32)
        def load32(d, shape, name):
            t = sb.tile(shape, fp32, tag=name)
            nc.sync.dma_start(t, d.ap())
            return t

        b1t = load32(d_b1t, [C, 4], "b1t")
        mb1t = load32(d_mb1t, [C, 4], "mb1t")
        bo_sb = load32(d_bo, [C, 1], "bo")
        ag_sb = load32(d_ag, [C, 1], "ag")
        ab_sb = load32(d_ab, [C, 1], "ab")
        ng_sb = load32(d_ng, [C, 1], "ng")
        nb_sb = load32(d_nb, [C, 1], "nb")

        # broadcast-along-partition tiles
        b2_bc = sb.tile([C, C], fp32)
        nc.sync.dma_start(b2_bc, bcast_ap(d_b2))
        mb2_bc = sb.tile([C, C], fp32)
        nc.sync.dma_start(mb2_bc, bcast_ap(d_mb2))

        # fold LN gains into projection weights:  wq' = diag(attn_g) @ wq
        def fold_w(d_w, g_vec, cols, name):
            t32 = scr.tile([C, cols], fp32, tag="ldtmp")
            nc.sync.dma_start(t32, d_w.ap())
            wfold = sb.tile([C, cols], bf16, tag=name)
            nc.vector.tensor_scalar_mul(wfold, t32, g_vec)
            wraw = scr.tile([C, cols], bf16, tag="wtmp")
            nc.vector.tensor_copy(wraw, t32)
            return wfold, wraw

        wq, wq_raw = fold_w(d_wq, ag_sb, HID, "wq")
        wk, wk_raw = fold_w(d_wk, ag_sb, HID, "wk")
        wv, wv_raw = fold_w(d_wv, ag_sb, HID, "wv")
        mw1, mw1_raw = fold_w(d_mw1, ng_sb, MLP_H, "mw1")

        ab_bf = sb.tile([C, 1], bf16)
        nc.vector.tensor_copy(ab_bf, ab_sb)
        nb_bf = sb.tile([C, 1], bf16)
        nc.vector.tensor_copy(nb_bf, nb_sb)

        # LN bias rows: bq = attn_b^T @ wq etc.  PSUM -> DRAM -> per-partition
        bq2 = sb.tile([C, 2], fp32)        # bq2[p, g] = bq[128 g + p]
        bk2 = sb.tile([C, 2], fp32)
        bv_bc = sb.tile([C, HID], fp32)    # broadcast along partitions
        bm1_t = sb.tile([C, 4], fp32)
        with tc.tile_pool(name="pbias", bufs=2, space="PSUM") as pb, \
             tc.tile_pool(name="dbias", bufs=2, space="DRAM") as db:
            for wraw, cols, dst, dst_ap in (
                (wq_raw, HID, bq2, [[1, C], [C, 2]]),
                (wk_raw, HID, bk2, [[1, C], [C, 2]]),
                (wv_raw, HID, bv_bc, [[0, C], [1, HID]]),
                (mw1_raw, MLP_H, bm1_t, [[1, C], [C, 4]]),
            ):
                bvec = nb_bf if wraw is mw1_raw else ab_bf
                bp = pb.tile([1, cols], fp32, tag="biasps")
                nc.tensor.matmul(bp, bvec, wraw, start=True, stop=True)
                bs = scr.tile([1, cols], fp32, tag="biassb")
                nc.vector.tensor_copy(bs, bp)
                dr = db.tile([1, cols], fp32, tag="biasdr")
                nc.sync.dma_start(dr, bs)
                nc.sync.dma_start(
                    dst, bass.AP(tensor=dr.tensor, offset=dr.offset, ap=dst_ap))
        mbias = sb.tile([C, 4], fp32)
        nc.vector.tensor_tensor(mbias, mb1t, bm1_t, ALU.add)

        # constants
        ident = sb.tile([C, C], bf16)
        make_identity(nc, ident)
        ones1 = sb.tile([C, 1], bf16)
        nc.vector.memset(ones1, 1.0)
        zrow = sb.tile([1, 512], bf16)
        nc.vector.memset(zrow, 0.0)
        zcol = sb.tile([1, C], bf16)
        nc.vector.memset(zcol, 0.0)
        eps_t = sb.tile([C, 1], fp32)
        nc.vector.memset(eps_t, EPS)
        rzfull = sb.tile([C, 512], bf16)
        nc.vector.memset(rzfull, 0.0)

        # ------------- load frame row -------------
        frow = sb.tile([1, N], bf16)
        for ch in range(4):
            ldfr = scr.tile([1, 1024], fp32, tag="ldfr")
            nc.sync.dma_start(ldfr, d_frow.ap()[0:1, ch * 1024:(ch + 1) * 1024])
            nc.vector.tensor_copy(frow[0:1, ch * 1024:(ch + 1) * 1024], ldfr)

        # ------------- fusion MLP (full batch, bf16, streamed) -------------
        xs_tok = sb.tile([C, 32, C], fp32)        # fused out, token-major

        with tc.tile_pool(name="fus1", bufs=2, space="PSUM") as fp1, \
             tc.tile_pool(name="fus2", bufs=3, space="PSUM") as fp2:
            for ch in range(4):
                ldx = scr.tile([C, 1024], fp32, tag="ldx")
                nc.sync.dma_start(ldx, d_xfm.ap()[:, ch * 1024:(ch + 1) * 1024])
                xfmc = scr.tile([C, 1024], bf16, tag="xfmc")
                nc.vector.tensor_copy(xfmc, ldx)
                hch = scr.tile([C, 4, 1024], bf16, tag="hch")
                for mh in range(4):
                    hp = fp1.tile([C, 1024], fp32, tag="h1p")
                    for nh in range(2):
                        sl = slice(nh * 512, (nh + 1) * 512)
                        fsl = slice(ch * 1024 + nh * 512,
                                    ch * 1024 + (nh + 1) * 512)
                        nc.tensor.matmul(hp[:, sl],
                                         w1a[:, mh * 128:(mh + 1) * 128],
                                         xfmc[:, sl], start=True, stop=False)
                        nc.tensor.matmul(hp[:, sl],
                                         w1b[0:1, mh * 128:(mh + 1) * 128],
                                         frow[0:1, fsl],
                                         start=False, stop=True)
                    nc.scalar.activation(hch[:, mh, :], hp, AF.Gelu,
                                         bias=b1t[:, mh:mh + 1], scale=1.0)
                for tbl in range(8):
                    tb = ch * 8 + tbl
                    h2p = fp2.tile([C, C], fp32, tag="h2p")
                    for mh in range(4):
                        nc.tensor.matmul(
                            h2p, hch[:, mh, tbl * 128:(tbl + 1) * 128],
                            w2[:, mh, :], start=(mh == 0), stop=(mh == 3))
                    nc.vector.tensor_tensor(xs_tok[:, tb, :], h2p, b2_bc,
                                            ALU.add)

        # ------------- LayerNorm (token-major), gains pre-folded -------------
        def layernorm(src, n_tiles, dst_bf):
            stats = scr.tile([C, n_tiles, 6], fp32, tag="lnstats")
            mv = scr.tile([C, n_tiles, 2], fp32, tag="lnmv")
            for tb in range(n_tiles):
                nc.vector.bn_stats(stats[:, tb, :], src[:, tb, :])
                nc.vector.bn_aggr(mv[:, tb, :], stats[:, tb, :])
            rstd = scr.tile([C, n_tiles], fp32, tag="lnrstd")
            nc.scalar.activation(rstd, mv[:, :, 1], AF.Sqrt,
                                 bias=eps_t, scale=1.0)
            nc.vector.reciprocal(rstd, rstd)
            for tb in range(n_tiles):
                nc.vector.tensor_scalar(
                    dst_bf[:, tb, :], src[:, tb, :],
                    mv[:, tb, 0:1], rstd[:, tb:tb + 1],
                    op0=ALU.subtract, op1=ALU.mult)

        xn_bf = sb.tile([C, 32, C], bf16, tag="xn_bf")
        layernorm(xs_tok, 32, xn_bf)

        xnT = sb.tile([C, N], bf16)               # feature-major LN1 out
        with tc.tile_pool(name="ptr", bufs=3, space="PSUM") as ptr:
            for tb in range(32):
                pt = ptr.tile([C, C], bf16, tag="tp")
                nc.tensor.transpose(pt, xn_bf[:, tb, :], ident)
                nc.vector.tensor_copy(xnT[:, tb * 128:(tb + 1) * 128], pt)

        # ------------- QKV projections -------------
        QT = sb.tile([C, 2, OWN], bf16)           # [4h x 32d, g, own token]
        KT = sb.tile([C, 2, N], bf16)
        V_tok = sb.tile([C, 32, HID], bf16)       # token-major V

        with tc.tile_pool(name="pqkv", bufs=2, space="PSUM") as pq:
            for g in range(2):
                qp = pq.tile([C, 1024], fp32, tag="qkp")
                for nh in range(2):
                    sl = slice(nh * 512, (nh + 1) * 512)
                    nc.tensor.matmul(qp[:, sl], wq[:, g * 128:(g + 1) * 128],
                                     xnT[:, sl], start=True, stop=True)
                nc.vector.tensor_scalar_add(QT[:, g, :], qp, bq2[:, g:g + 1])
                for nb in range(4):
                    kp = pq.tile([C, 1024], fp32, tag="qkp")
                    for nh in range(2):
                        sl = slice(nh * 512, (nh + 1) * 512)
                        fsl = slice(nb * 1024 + nh * 512,
                                    nb * 1024 + (nh + 1) * 512)
                        nc.tensor.matmul(kp[:, sl],
                                         wk[:, g * 128:(g + 1) * 128],
                                         xnT[:, fsl], start=True, stop=True)
                    nc.vector.tensor_scalar_add(
                        KT[:, g, nb * 1024:(nb + 1) * 1024], kp,
                        bk2[:, g:g + 1])
            for tb in range(32):
                vp = pq.tile([C, HID], fp32, tag="vp")
                nc.tensor.matmul(vp, xnT[:, tb * 128:(tb + 1) * 128], wv,
                                 start=True, stop=True)
                nc.vector.tensor_tensor(V_tok[:, tb, :], vp, bv_bc, ALU.add)

        # ------------- attention -------------
        xs2_tok = sb.tile([C, 8, C], fp32)        # own tokens: xs + attn_out

        if variant == "noattn":
            for tb in range(8):
                nc.vector.tensor_copy(xs2_tok[:, tb, :], xs_tok[:, tb, :])
        if variant != "noattn":
         p4pool = S.enter_context(tc.tile_pool(name="p4pool", bufs=3))
         with tc.tile_pool(name="ps_s", bufs=2, space="PSUM") as psS, \
             tc.tile_pool(name="ps_ot", bufs=1, space="PSUM") as psOT, \
             tc.tile_pool(name="ps_z", bufs=1, space="PSUM") as psZ, \
             tc.tile_pool(name="ps_m", bufs=2, space="PSUM") as psM:
             for ib in range(2):
                 onorm = [None, None]
                 for g in range(2):
                     ot = psOT.tile([C, 512], fp32, tag="ot")
                     zt = psZ.tile([C, 512], fp32, tag="zt")
                     # zero-init both banks with a single whole-bank matmul so
                     # the 4 interleaved col-group chains can accumulate with
                     # start=False (start=True clears has_written bank-wide)
                     nc.tensor.matmul(ot, zcol, zrow, start=True, stop=False,
                                      skip_group_check=True)
                     nc.tensor.matmul(zt, zcol, zrow, start=True, stop=False,
                                      skip_group_check=True)

                     def emit_avz(p4, jt):
                         last = (jt == 31)
                         for h4 in range(4):
                             nc.tensor.matmul(
                                 ot[32 * h4:32 * (h4 + 1), :],
                                 V_tok[:, jt, 32 * (4 * g + h4):
                                       32 * (4 * g + h4 + 1)],
                                 p4[:, h4 * 512:(h4 + 1) * 512],
                                 start=False, stop=(last and h4 == 3),
                                 tile_position=(0, 32 * h4),
                                 skip_group_check=True)
                             nc.tensor.matmul(
                                 zt[32 * h4:32 * h4 + 1, :],
                                 ones1,
                                 p4[:, h4 * 512:(h4 + 1) * 512],
                                 start=False, stop=(last and h4 == 3),
                                 tile_position=(0, 32 * h4),
                                 skip_group_check=True)

                     # software-pipelined: AV/Z for jt-1 are emitted after
                     # QK/exp for jt, so the PE never waits on the current
                     # tile's exp before starting the next tile's QK
                     prev = None
                     for jt in range(32):
                         p4 = p4pool.tile([C, 2048], bf16, tag="p4")
                         for half in range(2):
                             sps = psS.tile([C, 1024], fp32, tag="s")
                             for hh in range(2):
                                 h4 = half * 2 + hh   # head index in group
                                 nc.tensor.matmul(
                                     sps[:, hh * 512:(hh + 1) * 512],
                                     KT[32 * h4:32 * (h4 + 1), g,
                                        jt * 128:(jt + 1) * 128],
                                     QT[32 * h4:32 * (h4 + 1), g,
                                        ib * 512:(ib + 1) * 512],
                                     start=True, stop=True,
                                     tile_position=(32 * h4, 0))
                             nc.scalar.activation(
                                 p4[:, half * 1024:(half + 1) * 1024],
                                 sps, AF.Exp, scale=SCALE)
                         if prev is not None:
                             emit_avz(*prev)
                         prev = (p4, jt)
                     emit_avz(*prev)
                     # normalize: o / Z
                     with nc.allow_low_precision(reason="1/Z in bf16 is fine"):
                         for h4 in range(4):
                             nc.vector.reciprocal(
                                 rzfull[32 * h4:32 * h4 + 1, :],
                                 zt[32 * h4:32 * h4 + 1, :])
                     rzb = psM.tile([C, 512], fp32, tag="m")
                     nc.tensor.matmul(rzb, ind, rzfull, start=True, stop=True)
                     o_bf = scr.tile([C, 512], bf16, tag="obf")
                     nc.vector.tensor_copy(o_bf, ot)
                     og = scr.tile([C, 512], bf16, tag=f"onorm{g}")
                     nc.vector.tensor_tensor(og, o_bf, rzb, ALU.mult)
                     onorm[g] = og
                 # out-projection + bo
                 ao = psM.tile([C, 512], fp32, tag="m")
                 for g in range(2):
                     nc.tensor.matmul(ao, wo[:, g, :], onorm[g],
                                      start=(g == 0), stop=(g == 1))
                 aout = scr.tile([C, 512], bf16, tag="aout")
                 nc.vector.tensor_scalar_add(aout, ao, bo_sb)
                 # transpose to token-major + residual
                 for tt in range(4):
                     pt = psM.tile([C, C], bf16, tag="m")
                     nc.tensor.transpose(pt, aout[:, tt * 128:(tt + 1) * 128],
                                         ident)
                     tb = ib * 4 + tt
                     nc.vector.tensor_tensor(xs2_tok[:, tb, :], pt,
                                             xs_tok[:, tb, :], ALU.add)

        # ------------- LN2 + post-MLP (own tokens) -------------
        xn2_bf = sb.tile([C, 8, C], bf16, tag="xn2_bf")
        layernorm(xs2_tok, 8, xn2_bf)
        xn2T = sb.tile([C, OWN], bf16)
        with tc.tile_pool(name="ptr2", bufs=3, space="PSUM") as ptr2:
            for tb in range(8):
                pt = ptr2.tile([C, C], bf16, tag="tp2")
                nc.tensor.transpose(pt, xn2_bf[:, tb, :], ident)
                nc.vector.tensor_copy(xn2T[:, tb * 128:(tb + 1) * 128], pt)

        out_sb = sb.tile([C, 8, C], fp32)
        hm = sb.tile([C, 4, OWN], bf16, tag="hm")
        with tc.tile_pool(name="pmlp", bufs=2, space="PSUM") as pm, \
             tc.tile_pool(name="pmlp2", bufs=3, space="PSUM") as pm2:
            for mh in range(4):
                hp = pm.tile([C, OWN], fp32, tag="hmp")
                for nh in range(2):
                    sl = slice(nh * 512, (nh + 1) * 512)
                    nc.tensor.matmul(hp[:, sl],
                                     mw1[:, mh * 128:(mh + 1) * 128],
                                     xn2T[:, sl], start=True, stop=True)
                nc.scalar.activation(hm[:, mh, :], hp, AF.Gelu,
                                     bias=mbias[:, mh:mh + 1], scale=1.0)
            for tb in range(8):
                h2p = pm2.tile([C, C], fp32, tag="h2p2")
                for mh in range(4):
                    nc.tensor.matmul(h2p, hm[:, mh, tb * 128:(tb + 1) * 128],
                                     mw2[:, mh, :],
                                     start=(mh == 0), stop=(mh == 3))
                tmp = scr.tile([C, C], fp32, tag="otmp")
                nc.vector.tensor_tensor(tmp, h2p, mb2_bc, ALU.add)
                nc.vector.tensor_tensor(out_sb[:, tb, :], tmp,
                                        xs2_tok[:, tb, :], ALU.add)

        # ------------- store -------------
        oap = d_out.ap()
        nc.sync.dma_start(
            bass.AP(tensor=oap.tensor, offset=0,
                    ap=[[C, C], [C * C, 8], [1, C]]),
            out_sb)

    nc.compile()
    return nc


@functools.cache
def _get_nc(rep=1):
    return _build(rep)


def _prep_inputs(inputs):
    x = np.asarray(inputs["x"], np.float32)
    frame = np.asarray(inputs["frame_idx"], np.float32)
    # token order n = hw*T + t ; feature-major [C, N] per batch
    xb = x.reshape(B, C, T, HW).transpose(0, 1, 3, 2).reshape(B, C, N)
    xb = np.ascontiguousarray(xb)
    frow = np.ascontiguousarray(np.tile(frame, HW))[None, :]  # [1, N]

    def ktile(w, k):   # [k*128, C] -> [128, k, C]
        w = np.asarray(w, np.float32)
        return np.ascontiguousarray(w.reshape(k, 128, C).transpose(1, 0, 2))

    ind = np.zeros((C, C), np.float32)
    for p in range(C):
        ind[32 * (p // 32), p] = 1.0

    w1 = np.asarray(inputs["fusion_w1"], np.float32)
    common = {
        "frow": frow,
        "w1a": np.ascontiguousarray(w1[:C]),
        "w1b": np.ascontiguousarray(w1[C:C + 1]),
        "b1t": np.ascontiguousarray(
            np.asarray(inputs["fusion_b1"], np.float32).reshape(4, 128).T),
        "w2": ktile(inputs["fusion_w2"], 4),
        "b2": np.asarray(inputs["fusion_b2"], np.float32)[None, :],
        "attn_g": np.asarray(inputs["attn_norm_g"], np.float32)[:, None],
        "attn_b": np.asarray(inputs["attn_norm_b"], np.float32)[:, None],
        "wq": np.asarray(inputs["wq"], np.float32),
        "wk": np.asarray(inputs["wk"], np.float32),
        "wv": np.asarray(inputs["wv"], np.float32),
        "wo": ktile(inputs["wo"], 2),
        "bo": np.asarray(inputs["bo"], np.float32)[:, None],
        "norm_g": np.asarray(inputs["norm_g"], np.float32)[:, None],
        "norm_b": np.asarray(inputs["norm_b"], np.float32)[:, None],
        "mw1": np.asarray(inputs["mlp_w1"], np.float32),
        "mb1t": np.ascontiguousarray(
            np.asarray(inputs["mlp_b1"], np.float32).reshape(4, 128).T),
        "mw2": ktile(inputs["mlp_w2"], 4),
        "mb2": np.asarray(inputs["mlp_b2"], np.float32)[None, :],
        "ind128": ind,
    }
    common = {k: np.ascontiguousarray(v) for k, v in common.items()}

    in_maps = []
    for c in range(NCORES):
        b, q = c // 4, c % 4
        m = dict(common)
        m["xfm"] = np.ascontiguousarray(np.roll(xb[b], -OWN * q, axis=1))
        in_maps.append(m)
    return in_maps


def _make_runner(nc):
    """Build a per-device jit runner for a program (no shard_map: the
    8-way shard_map execute path deadlocks on the axon tunnel)."""
    import jax
    from concourse import bass2jax, mybir

    bass2jax.install_neuronx_cc_hook()

    in_names, out_names, out_avals, zero_outs = [], [], [], []
    for alloc in nc.m.functions[0].allocations:
        if not isinstance(alloc, mybir.MemoryLocationSet):
            continue
        name = alloc.memorylocations[0].name
        if alloc.kind == "ExternalInput":
            in_names.append(name)
        elif alloc.kind == "ExternalOutput":
            out_names.append(name)
            shape = tuple(alloc.tensor_shape)
            dtype = mybir.dt.np(alloc.dtype)
            out_avals.append(jax.core.ShapedArray(shape, dtype))
            zero_outs.append(np.zeros(shape, dtype))
    n_params = len(in_names)

    def _body(*args):
        return tuple(bass2jax._bass_exec_p.bind(
            *args,
            out_avals=tuple(out_avals),
            in_names=tuple(in_names + out_names),
            out_names=tuple(out_names),
            lowering_input_output_aliases=(),
            sim_require_finite=True,
            sim_require_nnan=True,
            nc=nc,
        ))

    donate = tuple(range(n_params, n_params + len(out_names)))
    jf = jax.jit(_body, donate_argnums=donate, keep_unused=True)
    return jf, in_names, out_names, zero_outs


@functools.cache
def _get_runner():
    return _make_runner(_get_nc())


def _run_spmd(in_maps):
    import jax

    jf, in_names, out_names, zero_outs = _get_runner()
    devs = jax.devices()[:NCORES]
    results = []
    for i, d in enumerate(devs):
        vals = dict(in_maps[i])
        vals.setdefault("partition_id", np.array([[i]], np.uint32))
        ins = [jax.device_put(np.asarray(vals[n]), d) for n in in_names]
        zs = [jax.device_put(z, d) for z in zero_outs]
        out = jf(*ins, *zs)
        results.append(
            {name: np.asarray(out[k]) for k, name in enumerate(out_names)})
    return results


def kernel(**inputs):
    in_maps = _prep_inputs(inputs)
    results = _run_spmd(in_maps)

    xs_full = np.zeros((B, N, C), np.float32)
    for c in range(NCORES):
        b, q = c // 4, c % 4
        xs_full[b, OWN * q:OWN * (q + 1), :] = results[c]["out"]
    out = xs_full.reshape(B, HW, T, C).transpose(0, 3, 2, 1)
    return np.ascontiguousarray(out.reshape(B, C, T, H, W))

